# revision 27
# baseline (speedup 1.0000x reference)
"""Multi-head self-attention (B=4, S=2048, E=1024, H=16, causal) on 8 TRN2
NeuronCores, tensor-parallel over heads (2 heads/core).

v4: QK row-pairing + baseline ones-column PV + scheduling fixes.
  - Attention in 1-k-block groups (both heads): QK^T row-paired (h0 rows
    0-63 / h1 rows 64-127 of the PE array, concurrent — genuine 2-in-1
    overlap, measured), ONE [128,1024] PSUM tile and ONE exp per group.
  - PV per head with the 65-col ones-column trick (sums ride the PV
    stream for free): v_sb [128, 64, 130] = [1|v_h0|1|v_h1] so each
    head's lhsT [128,65] has its ones col first -> att row 0 = sums,
    rows 1-64 = attn dims. One strided cast per v-proj sub-block.
  - exp is restricted past the fully-masked prefix of diagonal blocks
    (DVE memsets the prefix); causal mask multiply touches only the
    128-col boundary (one m-independent triangle constant).
  - att-bank casts run on GpSimd (idle engine) so the next unit's PSUM
    frees while ACT/PE start its QK/exp pipeline.
  - Projection is a quantum stream (qt DMA lookahead, q/k chunk MMs,
    epilogues on DVE, sub-major v chains) interleaved into the attention
    groups; ~half is deferred into the odd pass to keep the PE dense.
Phase 3 (A2A re-shard + normalize + output projection) as v1; output DMA
split per 2 m-chunks; odd pass ends with a small unit so the tail A2A
starts early.
"""
import sys

if "/opt/trn_rl_repo" not in sys.path:
    sys.path.insert(0, "/opt/trn_rl_repo")

import numpy as np
import ml_dtypes

BF16 = ml_dtypes.bfloat16

B, S, E, H, D = 4, 2048, 1024, 16, 64
T = B * S  # 8192
N_CORES = 8
HPC = H // N_CORES  # 2 heads per core
TL = T // N_CORES  # 1024 tokens per core for the output shard
NTB = T // 512  # 16 projection t-blocks
SCALE = 1.0 / np.sqrt(D)

_CACHE = {}


def build_kernel():
    import concourse.mybir as mybir
    import concourse.tile as tile
    from concourse import bacc
    from concourse.bass import ds, ts, _add_dep_helper

    F32 = mybir.dt.float32
    BF = mybir.dt.bfloat16
    AF = mybir.ActivationFunctionType
    ALU = mybir.AluOpType

    nc = bacc.Bacc("TRN2", target_bir_lowering=False, debug=False,
                   num_devices=N_CORES)

    qT_d = nc.dram_tensor("qT", [E, T], BF, kind="ExternalInput")
    wqk_d = nc.dram_tensor("wqk", [E, 256], BF, kind="ExternalInput")
    wv_d = nc.dram_tensor("wv", [E, 128], BF, kind="ExternalInput")
    bqk_d = nc.dram_tensor("bqk", [128, 2], F32, kind="ExternalInput")
    wout_d = nc.dram_tensor("wout", [E, E], BF, kind="ExternalInput")
    bout_d = nc.dram_tensor("bout", [128, 8], F32, kind="ExternalInput")
    # m-independent boundary triangle, duplicated per head: [p, h*128+q]
    masks_d = nc.dram_tensor("masks", [128, 256], BF, kind="ExternalInput")
    sel_d = nc.dram_tensor("sel", [16, 8, 128], BF, kind="ExternalInput")
    outT_d = nc.dram_tensor("outT", [E, TL], F32, kind="ExternalOutput")

    with tile.TileContext(nc) as tc:
        with (
            tc.tile_pool(name="consts", bufs=1) as cpool,
            tc.tile_pool(name="dram", bufs=1, space="DRAM") as dram,
            tc.tile_pool(name="qk", bufs=2, space="PSUM") as ps_qk,
            tc.tile_pool(name="proj", bufs=2, space="PSUM") as ps_proj,
            tc.tile_pool(name="att", bufs=2, space="PSUM") as ps_att,
            tc.tile_pool(name="persist", bufs=1) as ppool,
            tc.tile_pool(name="qt", bufs=2) as qtpool,
            tc.tile_pool(name="pex", bufs=3) as ppex,
            tc.tile_pool(name="ph3", bufs=2) as p3,
        ):
            # ---- constants needed by the first projection blocks go first
            wqk_sb = cpool.tile([128, 8, 256], BF)
            nc.sync.dma_start(wqk_sb[:], wqk_d.ap().rearrange("(c p) f -> p c f", p=128))

            q_sb = ppool.tile([128, T], BF, tag="q_sb")
            k_sb = ppool.tile([128, T], BF, tag="k_sb")
            # [1|v_h0|1|v_h1] per token-block: head h lhsT = cols [65h, 65h+65)
            v_sb = ppool.tile([128, 64, 130], BF, tag="v_sb")
            # row 0 = softmax sums, rows 1-64 = unnormalized attn dims
            attnU = ppool.tile([65, 2, T], BF, tag="attnU")

            qT_r = qT_d.ap().rearrange("(c p) t -> p c t", p=128)

            def emit_qt_dma(tb, ways=2):
                qt = qtpool.tile([128, 8, 512], BF, name="qt")
                w = 8 // ways
                for i in range(ways):
                    nc.sync.dma_start(qt[:, i * w:(i + 1) * w, :],
                                      qT_r[:, i * w:(i + 1) * w, ts(tb, 512)])
                return qt

            qt0 = emit_qt_dma(0, ways=4)
            bqk_sb = cpool.tile([128, 2], F32)
            nc.sync.dma_start(bqk_sb[:], bqk_d.ap())
            wv_sb = cpool.tile([128, 8, 128], BF)
            nc.sync.dma_start(wv_sb[:], wv_d.ap().rearrange("(c p) f -> p c f", p=128))
            qt1 = emit_qt_dma(1, ways=4)

            nc.vector.memset(v_sb[:, :, 0:1], 1.0)
            nc.vector.memset(v_sb[:, :, 65:66], 1.0)

            # ---- projection quantum stream --------------------------------
            class ProjStream:
                def __init__(self, qt0, qt1):
                    self.queue = []  # pending quanta (callables)
                    self.done_tb = -1  # highest tb fully EMITTED
                    self.next_tb = 0
                    self._next_qt = qt0
                    self._lookahead_qt = qt1  # block 1's tile, already loading

                def _load_block(self, tb):
                    qt = self._next_qt
                    self._next_qt = self._lookahead_qt
                    self._lookahead_qt = None
                    state = {}

                    def dma():
                        if tb + 1 < NTB and self._next_qt is None:
                            self._next_qt = emit_qt_dma(tb + 1)

                    def qk_chunk(c):
                        if c == 0:
                            state["pq"] = ps_proj.tile([128, 512], F32,
                                                       tag="proj", name="pq")
                            state["pk"] = ps_proj.tile([128, 512], F32,
                                                       tag="proj", name="pk")
                        nc.tensor.matmul(state["pq"][:], wqk_sb[:, c, 0:128],
                                         qt[:, c, :],
                                         start=(c == 0), stop=(c == 7))
                        nc.tensor.matmul(state["pk"][:], wqk_sb[:, c, 128:256],
                                         qt[:, c, :],
                                         start=(c == 0), stop=(c == 7))

                    def epi():
                        nc.vector.tensor_scalar_add(q_sb[:, ts(tb, 512)],
                                                    state["pq"][:], bqk_sb[:, 0:1])
                        nc.vector.tensor_scalar_add(k_sb[:, ts(tb, 512)],
                                                    state["pk"][:], bqk_sb[:, 1:2])

                    def v_sub(s):
                        pv = ps_proj.tile([128, 128], F32, tag="proj", name="pv")
                        for c in range(8):
                            nc.tensor.matmul(pv[:], qt[:, c, ds(s * 128, 128)],
                                             wv_sb[:, c, :], start=(c == 0),
                                             stop=(c == 7))
                        # psum [128, 2, 64] -> v cols {1-64, 66-129}
                        dst = v_sb[:, tb * 4 + s, :].rearrange(
                            "p (h w) -> p h w", h=2)[:, :, 1:65]
                        src = pv[:].rearrange("p (h w) -> p h w", h=2)
                        nc.vector.tensor_copy(dst, src)

                    def last():
                        self.done_tb = tb

                    q = [dma]
                    q += [lambda c=c: qk_chunk(c) for c in range(8)]
                    q += [epi]
                    q += [lambda s=s: v_sub(s) for s in range(4)]
                    q += [last]
                    self.queue.extend(q)

                def fill(self, n):
                    for _ in range(n):
                        if not self.queue:
                            if self.next_tb >= NTB:
                                return
                            self._load_block(self.next_tb)
                            self.next_tb += 1
                        self.queue.pop(0)()

                def ensure(self, tb):
                    while self.done_tb < min(tb, NTB - 1):
                        self.fill(1)

                def drain(self):
                    self.ensure(NTB - 1)

            proj = ProjStream(qt0, qt1)
            # blocks 0 and 1 fully, before attention (their q/k/v are deps)
            proj.ensure(1)

            # ---- deferred constants (used from attention / phase 3 on)
            masks_sb = cpool.tile([128, 256], BF)
            nc.sync.dma_start(masks_sb[:], masks_d.ap())
            sel_sb = cpool.tile([16, 8, 128], BF)
            nc.sync.dma_start(sel_sb[:], sel_d.ap())
            wout_sb = cpool.tile([128, 8, 1024], BF)
            nc.sync.dma_start(wout_sb[:], wout_d.ap().rearrange("(c p) e -> p c e", p=128))
            bout_sb = cpool.tile([128, 8], F32)
            nc.sync.dma_start(bout_sb[:], bout_d.ap())

            def attention_unit(b, j, rate=(2, 1)):
                q0 = b * S + j * 512
                nkb = 4 * j + 4
                att = [ps_att.tile([65, 512], F32, tag="att", name=f"att{hh}")
                       for hh in range(2)]
                last_exp = last_pv = None

                def emit_qk(kb):
                    k0 = b * S + kb * 128
                    qkt = ps_qk.tile([128, 1024], F32, tag="qk", name="qkt")
                    for hh in range(2):
                        nc.tensor.matmul(
                            qkt[:, ds(hh * 512, 512)],
                            k_sb[ds(hh * 64, 64), ds(k0, 128)],
                            q_sb[ds(hh * 64, 64), ds(q0, 512)],
                            start=True, stop=True)
                    return qkt

                qkt = emit_qk(0)
                for kb in range(nkb):
                    t128 = b * 16 + kb
                    m = kb - 4 * j
                    pb = ppex.tile([128, 1024], BF, name="pb")
                    pb_r = pb[:].rearrange("p (h q) -> p h q", h=2)
                    if m >= 1:
                        # zero the fully-masked prefix (exp skips it);
                        # GpSimd is idle and pb is SBUF
                        nc.gpsimd.memset(pb_r[:, :, 0:128 * m], 0.0)
                        last_exp = nc.scalar.activation(
                            pb_r[:, :, 128 * m:512],
                            qkt[:].rearrange("p (h q) -> p h q", h=2)[:, :, 128 * m:512],
                            AF.Exp, scale=SCALE)
                    else:
                        last_exp = nc.scalar.activation(pb[:], qkt[:], AF.Exp,
                                                        scale=SCALE)
                    # next group's QK enters the PE queue before this
                    # group's exp-dependent PV so the PE never waits on ACT
                    if kb + 1 < nkb:
                        qkt = emit_qk(kb + 1)
                    if m >= 0:
                        # causal boundary: q in [128m, 128m+128)
                        mk = masks_sb[:].rearrange("p (h q) -> p h q", h=2)
                        nc.vector.tensor_tensor(
                            pb_r[:, :, ds(128 * m, 128)],
                            pb_r[:, :, ds(128 * m, 128)],
                            mk[:, :, :], op=ALU.mult)
                    proj.fill(rate[kb % 2])
                    for hh in range(2):
                        last_pv = nc.tensor.matmul(
                            att[hh][:], v_sb[:, t128, ds(65 * hh, 65)],
                            pb[:, ds(hh * 512, 512)],
                            start=(kb == 0), stop=(kb == nkb - 1))
                # row 0 = sums, rows 1-64 = attn dims
                for hh in range(2):
                    last_cast = nc.vector.tensor_copy(
                        attnU[:, hh, ds(q0, 512)], att[hh][:])
                return last_exp, last_pv, last_cast

            a2a_in = [dram.tile([N_CORES, 130, 512], BF, tag=f"a2a_in{i}",
                                name=f"a2a_in{i}") for i in range(2)]
            a2a_out = [dram.tile([N_CORES, 130, 512], BF, tag=f"a2a_out{i}",
                                 name=f"a2a_out{i}") for i in range(2)]

            # dest core c's half-h tokens = global 512-block (2c + h)
            attnU_r = attnU[:, :, :].rearrange("p h (blk t) -> p h blk t", t=512)

            def stage_half(half, c0, c1):
                for hh in range(2):
                    nc.sync.dma_start(
                        a2a_in[half][c0:c1, ds(64 * hh, 64), :]
                        .rearrange("c p t -> p c t"),
                        attnU_r[1:65, hh, ds(2 * c0 + half, c1 - c0, 2), :])
                    nc.sync.dma_start(
                        a2a_in[half][c0:c1, 128 + hh, :],
                        attnU_r[0:1, hh, ds(2 * c0 + half, c1 - c0, 2), :])

            def stage_and_a2a(half, from_core=0):
                stage_half(half, from_core, N_CORES)
                nc.gpsimd.collective_compute(
                    "AllToAll", ALU.bypass,
                    replica_groups=[list(range(N_CORES))],
                    ins=[a2a_in[half][:].opt()], outs=[a2a_out[half][:].opt()])

            def phase3_prefetch(half):
                af = p3.tile([128, 8, 512], BF, tag="af", name="af")
                rsrc = p3.tile([16, 512], BF, tag="rsrc", name="rsrc")
                # sums first (reciprocal chain is ready when af lands); af
                # split across two queues to halve the load latency
                nc.sync.dma_start(rsrc[:], a2a_out[half][:, 128:130, :])
                nc.sync.dma_start(
                    af[:, 0:4, :],
                    a2a_out[half][0:4, 0:128, :].rearrange("c p t -> p c t"))
                nc.sync.dma_start(
                    af[:, 4:8, :],
                    a2a_out[half][4:8, 0:128, :].rearrange("c p t -> p c t"))
                return af, rsrc

            def phase3_compute(half, af, rsrc, gates):
                """gates: dict engine->BassInstruction the first op of that
                engine's queue must not be scheduled before."""
                def gate(inst, eng):
                    if gates.get(eng) is not None:
                        _add_dep_helper(inst.ins, gates[eng].ins, sync=False,
                                        reason="phase3 queue-order gate")
                    gates[eng] = None

                r32 = p3.tile([16, 512], F32, tag="r32", name="r32")
                rf32 = p3.tile([16, 512], F32, tag="rf32", name="rf32")
                rbf = p3.tile([16, 512], BF, tag="rbf", name="rbf")
                # approx reciprocal (~18 bits; sums are O(100..2500))
                gate(nc.vector.tensor_copy(r32[:], rsrc[:]), "v")
                nc.vector.reciprocal_approx_fast(rf32[:], r32[:])
                nc.vector.tensor_copy(rbf[:], rf32[:])
                last_tt = None
                for c in range(8):
                    rb = ps_att.tile([128, 512], F32, tag="att", name="rb")
                    gate(nc.tensor.matmul(rb[:], sel_sb[:, c, :], rbf[:],
                                          start=True, stop=True), "pe")
                    last_tt = nc.vector.tensor_tensor(af[:, c, :], af[:, c, :],
                                                      rb[:], op=ALU.mult)
                osb = p3.tile([128, 8, 512], F32, tag="osb", name="osb")
                outT_r = outT_d.ap().rearrange("(m p) t -> p m t", p=128)
                last_act = last_mm = None
                for m in range(8):
                    po = ps_proj.tile([128, 512], F32, tag="proj", name="po")
                    for c in range(8):
                        last_mm = nc.tensor.matmul(
                            po[:], wout_sb[:, c, ds(m * 128, 128)],
                            af[:, c, :], start=(c == 0), stop=(c == 7))
                    last_act = nc.scalar.activation(osb[:, m, :], po[:], AF.Identity,
                                                    bias=bout_sb[:, ds(m, 1)])
                    if m == 0:
                        gate(last_act, "s")
                    if m % 2 == 1:  # ship each finished pair of m-chunks
                        nc.sync.dma_start(outT_r[:, m - 1:m + 1, ts(half, 512)],
                                          osb[:, m - 1:m + 1, :])
                return {"v": last_tt, "s": last_act, "pe": last_mm}

            # ---- main schedule -------------------------------------------
            # Even pass must host nearly all of proj (unit (3,2) needs tb14):
            # fill 3/2 per group so the pre-unit ensure bursts stay small.
            for b in range(B):
                for j in (0, 2):
                    proj.ensure(b * 4 + j)
                    attention_unit(b, j, rate=(3, 2))
            stage_and_a2a(0)
            af0, rsrc0 = phase3_prefetch(0)
            le = lp = lc = None
            odd_order = [(0, 3), (0, 1), (1, 3), (1, 1), (2, 3), (2, 1),
                         (3, 3), (3, 1)]
            for b, j in odd_order:
                proj.ensure(b * 4 + j)
                le, lp, lc = attention_unit(b, j, rate=(1, 0))
                if (b, j) == (1, 1):
                    stage_half(1, 0, 4)
                if (b, j) == (2, 1):
                    stage_half(1, 4, 6)
                if (b, j) == (3, 3):
                    stage_half(1, 7, 8)  # core 7's odd half = block 15 = (3,3)
            proj.drain()
            stage_half(1, 6, 7)  # core 6's odd half = block 13 = (3,1)
            nc.gpsimd.collective_compute(
                "AllToAll", ALU.bypass,
                replica_groups=[list(range(N_CORES))],
                ins=[a2a_in[1][:].opt()], outs=[a2a_out[1][:].opt()])
            af1, rsrc1 = phase3_prefetch(1)
            lasts = phase3_compute(0, af0, rsrc0, {"v": lc, "s": le, "pe": lp})
            # keep the PE's HAM clock warm through the A2A-1 wait so
            # phase3(1) doesn't run at quarter clock (write-only scratch)
            warm = ps_att.tile([128, 512], F32, tag="att", name="warm")
            for i in range(24):
                nc.tensor.matmul(warm[:], sel_sb[:, i % 8, :], rsrc0[:],
                                 start=True, stop=True)
            phase3_compute(1, af1, rsrc1, lasts)

    nc.compile()
    return nc


def prep_inputs(query, w_in, b_in, w_out, b_out):
    """Shard + lay out host-side. Returns in_maps for the 8 cores."""
    query = np.asarray(query, dtype=np.float32)
    w_in = np.asarray(w_in, dtype=np.float32)
    b_in = np.asarray(b_in, dtype=np.float32)
    w_out = np.asarray(w_out, dtype=np.float32)
    b_out = np.asarray(b_out, dtype=np.float32)

    qT = np.ascontiguousarray(query.reshape(T, E).T).astype(BF16)
    woutT = np.ascontiguousarray(w_out.T).astype(BF16)
    b_v = b_in[2 * E:3 * E]
    bout_eff = (b_out + w_out @ b_v).reshape(8, 128).T.copy()  # [128, 8]

    # boundary triangle (m-independent): mask[p, h*128 + qq] = p <= qq
    qq = np.arange(128)[None, :]
    pidx = np.arange(128)[:, None]
    tri = (pidx <= qq)
    masks = np.concatenate([tri, tri], axis=1).astype(BF16)  # [128, 256]

    sel = np.zeros((16, 8, 128), dtype=BF16)
    for c in range(8):
        sel[2 * c, c, 0:64] = 1.0
        sel[2 * c + 1, c, 64:128] = 1.0

    in_maps = []
    for c in range(N_CORES):
        r = slice(128 * c, 128 * c + 128)
        wqk = np.concatenate([w_in[:E][r].T, w_in[E:2 * E][r].T], axis=1)
        wv = w_in[2 * E:3 * E][r].T
        bqk = np.stack([b_in[:E][r], b_in[E:2 * E][r]], axis=1)
        in_maps.append({
            "qT": qT,
            "wqk": np.ascontiguousarray(wqk).astype(BF16),
            "wv": np.ascontiguousarray(wv).astype(BF16),
            "bqk": np.ascontiguousarray(bqk),
            "wout": woutT,
            "bout": np.ascontiguousarray(bout_eff),
            "masks": masks,
            "sel": sel,
        })
    return in_maps


def run_on_hw(in_maps, trace=False, **kw):
    from concourse.bass_utils import run_bass_kernel_spmd

    if "nc" not in _CACHE:
        _CACHE["nc"] = build_kernel()
    return run_bass_kernel_spmd(_CACHE["nc"], in_maps, list(range(N_CORES)),
                                trace=trace, **kw)


def kernel(query, w_in, b_in, w_out, b_out):
    in_maps = prep_inputs(query, w_in, b_in, w_out, b_out)
    res = run_on_hw(in_maps)
    parts = [res.results[c]["outT"].T for c in range(N_CORES)]  # [TL, E] each
    out = np.concatenate(parts, axis=0).reshape(B, S, E)
    return out.astype(np.float32)
